# revision 13
# baseline (speedup 1.0000x reference)
"""Sparse-attention (graph-modulated MHA) Bass kernel for Trainium2.

Strategy: data-parallel over batch (8 batches -> 8 NeuronCores). Per core:
  - key-mask gather on HOST: only ~250 of 512 key positions are unmasked
    (pure key mask); k/v inputs are gathered to a static capacity KP=288
    and padded, cutting K-proj/V-proj/scores/AV matmul work ~44%
  - bf16 matmuls (fp32 psum); V projection first, then Q/K projections
    interleaved per head-pair with the score matmuls
  - scores computed transposed sT[k_pos, q]; the two heads of a pair share
    one [*, 1024] psum tile so one exp covers both; graph block (ones-padded
    to 128 gathered rows) multiplied on raw fp32 psum scores; pad-key mask
    folded into the exp bias
  - softmax without max-subtraction; denominator L from an extra ones-column
    in the attention*V matmul; 1/L broadcast across partitions via fp32r
    rank-1 PE matmuls
  - bv folded into bm on host (attention rows sum to 1), so the V-proj
    psum is packed into vha with a plain cast copy
  - merge projection emitted transposed (bf16 out); host transposes back
  - input DMAs spread round-robin over 4 engine queues (sync/gpsimd/
    scalar/vector) so the V/Q weight streams land ~2x faster at start
"""
import sys

sys.path.insert(0, "/opt/trn_rl_repo")

import ml_dtypes
import numpy as np

import concourse.bass as bass
import concourse.mybir as mybir
import concourse.tile as tile
from concourse import bacc, bass_utils
from concourse.bass import ds, ts

B, S, D, H, DK = 8, 512, 1024, 16, 64
GN = 100
P = 128
KP = 288                              # gathered-key capacity (max unmasked=258)
KCH = [(0, 128), (128, 128), (256, 32)]   # (offset, rows) gathered kpos chunks
NKC = len(KCH)
NDT = D // P      # 8 hidden chunks of 128
NPAIR = H // 2    # 8 head pairs (2 heads share a 128-partition tile)
EH = DK + 1       # head slot width in vha (64 v-cols + 1 ones col)
F32 = mybir.dt.float32
BF16 = mybir.dt.bfloat16
FT = mybir.ActivationFunctionType
ALU = mybir.AluOpType

_CACHE: dict = {}


def _build_module():
    nc = bacc.Bacc("TRN2", target_bir_lowering=False, debug=False)
    dram = {}
    dram["qinT"] = nc.dram_tensor("qinT", [D, S], BF16, kind="ExternalInput").ap()
    for nm in ("kinT", "vinT"):
        dram[nm] = nc.dram_tensor(nm, [D, KP], BF16, kind="ExternalInput").ap()
    for nm in ("wqT", "wkT", "wvT", "wmT"):
        dram[nm] = nc.dram_tensor(nm, [D, D], BF16, kind="ExternalInput").ap()
    for nm in ("bq", "bk", "bm"):
        dram[nm] = nc.dram_tensor(nm, [P, NDT], F32, kind="ExternalInput").ap()
    dram["maskb"] = nc.dram_tensor("maskb", [P, NKC], F32, kind="ExternalInput").ap()
    dram["gT"] = nc.dram_tensor("gT", [P, GN], F32, kind="ExternalInput").ap()
    outT = nc.dram_tensor("outT", [D, S], BF16, kind="ExternalOutput").ap()

    with tile.TileContext(nc) as tc:
        with (
            tc.tile_pool(name="wpool", bufs=24) as wpool,
            tc.tile_pool(name="vkpool", bufs=16) as vkpool,
            tc.tile_pool(name="qpool", bufs=8) as qpool,
            tc.tile_pool(name="qTpool", bufs=8) as qTpool,
            tc.tile_pool(name="kTpool", bufs=8) as kTpool,
            tc.tile_pool(name="vpool", bufs=3) as vpool,
            tc.tile_pool(name="ptpool", bufs=10) as ptpool,
            tc.tile_pool(name="opool", bufs=8) as opool,
            tc.tile_pool(name="outpool", bufs=3) as outpool,
            tc.tile_pool(name="cpool", bufs=1) as cpool,
            tc.tile_pool(name="rlpool", bufs=4) as rlpool,
            tc.tile_pool(name="rlbpool", bufs=4) as rlbpool,
            tc.tile_pool(name="ppsum", bufs=2, space="PSUM") as ppsum,
            tc.tile_pool(name="spsum", bufs=2, space="PSUM") as spsum,
            tc.tile_pool(name="apsum", bufs=2, space="PSUM") as apsum,
        ):
            E3 = [nc.sync, nc.gpsimd, nc.scalar]

            def load_chunks(name, width, pool, tag, engs, rot=0):
                tiles = []
                src = dram[name].rearrange("(t p) f -> t p f", p=P)
                for k_i in range(NDT):
                    t_ = pool.tile([P, width], BF16, tag=tag)
                    engs[(k_i + rot) % len(engs)].dma_start(t_[:], src[k_i])
                    tiles.append(t_)
                return tiles

            # PE warmup: full-duty N=512 matmuls on memset tiles while the
            # first DMAs land, so the HAM un-throttles before real matmuls
            warm_w = cpool.tile([P, DK], BF16, tag="warmw")
            nc.vector.memset(warm_w[:], 0.0)
            warm_x = cpool.tile([P, S], BF16, tag="warmx")
            nc.vector.memset(warm_x[:], 0.0)
            wps = apsum.tile([P, S], F32, tag="ap", name="warmps")
            for _ in range(10):
                nc.tensor.matmul(wps[0:DK, :], warm_w[:], warm_x[:], start=True, stop=True)

            # V inputs stream first (V projection runs first), 4-way spread
            wvt = load_chunks("wvT", D, wpool, "w", E3)
            vt = load_chunks("vinT", KP, vkpool, "vk", E3, rot=1)

            # ---- constants (gpsimd DMA queue; small) ----
            bqt = cpool.tile([P, NDT], F32, tag="bqt")
            nc.gpsimd.dma_start(bqt[:], dram["bq"])
            bkt = cpool.tile([P, NDT], F32, tag="bkt")
            nc.gpsimd.dma_start(bkt[:], dram["bk"])
            bmt = cpool.tile([P, NDT], F32, tag="bmt")
            nc.gpsimd.dma_start(bmt[:], dram["bm"])
            maskb = cpool.tile([P, NKC], F32, tag="maskb")
            nc.gpsimd.dma_start(maskb[:], dram["maskb"])
            gt = cpool.tile([P, GN], F32, tag="gt")
            nc.gpsimd.dma_start(gt[:], dram["gT"])
            ones64 = cpool.tile([1, DK], mybir.dt.float32r, tag="ones64")
            nc.vector.memset(ones64[:].bitcast(F32), 1.0)

            # Q/K inputs stream behind V
            wqt = load_chunks("wqT", D, wpool, "w", E3, rot=1)
            qt = load_chunks("qinT", S, qpool, "q", E3, rot=2)
            wkt = load_chunks("wkT", D, wpool, "w", E3)
            ktc = load_chunks("kinT", KP, vkpool, "vk", E3, rot=1)

            # ---- V projection (natural layout, packed into vha + ones col;
            #      bv folded into bm on host) ----
            vha = [vpool.tile([P, H * EH], BF16, tag="vha", name=f"vha{i}") for i in range(NKC)]
            for ci, (off, rows) in enumerate(KCH):
                v3 = vha[ci].rearrange("p (h e) -> p h e", e=EH)
                for half in range(2):
                    ps = ppsum.tile([P, S], F32, tag="pp")
                    for k_i in range(NDT):
                        nc.tensor.matmul(
                            ps[0:rows, :], vt[k_i][:, ds(off, rows)],
                            wvt[k_i][:, ts(half, 512)],
                            start=(k_i == 0), stop=(k_i == NDT - 1),
                        )
                    dst3 = v3[0:rows, half * 8 : half * 8 + 8, 0:DK]
                    src3 = ps[0:rows, :].rearrange("p (h d) -> p h d", d=DK)
                    nc.vector.tensor_copy(dst3, src3)
                nc.vector.memset(v3[0:rows, :, DK : DK + 1], 1.0)

            # merge weights stream during the attention phase
            wmt = load_chunks("wmT", D, wpool, "w", E3, rot=2)

            # ---- attention state ----
            oT = [opool.tile([P, S], BF16, tag="o", name=f"oT{i}") for i in range(NPAIR)]
            qT, kT = [None] * NPAIR, [None] * NPAIR

            def emit_qproj(m):
                ps = ppsum.tile([P, S], F32, tag="pp")
                for k_i in range(NDT):
                    nc.tensor.matmul(
                        ps[:], wqt[k_i][:, ts(m, P)], qt[k_i][:],
                        start=(k_i == 0), stop=(k_i == NDT - 1),
                    )
                t_ = qTpool.tile([P, S], BF16, tag="qT")
                nc.scalar.activation(t_[:], ps[:], FT.Identity, bias=bqt[:, m : m + 1])
                qT[m] = t_

            def emit_kproj(m):
                ps = ppsum.tile([P, S], F32, tag="pp")
                for k_i in range(NDT):
                    nc.tensor.matmul(
                        ps[:, 0:KP], wkt[k_i][:, ts(m, P)], ktc[k_i][:],
                        start=(k_i == 0), stop=(k_i == NDT - 1),
                    )
                t_ = kTpool.tile([P, KP], BF16, tag="kT")
                nc.vector.tensor_scalar_add(t_[:], ps[:, 0:KP], bkt[:, m : m + 1])
                kT[m] = t_

            def emit_scores(t):
                """Both heads of pair t share one [*, 2*S] psum tile per k-chunk."""
                tiles = [None] * NKC
                for ci, (off, rows) in enumerate(KCH):
                    sps = spsum.tile([P, 2 * S], F32, tag="sp")
                    for x in range(2):
                        nc.tensor.matmul(
                            sps[0:rows, ts(x, S)],
                            kT[t][x * DK : (x + 1) * DK, ds(off, rows)],
                            qT[t][x * DK : (x + 1) * DK, :],
                            start=True, stop=True,
                        )
                        if ci == 0:
                            # gathered kpos<100 are the first n_g<=52 rows;
                            # host pads gT with ones rows so full-128 mult is safe
                            nc.vector.tensor_tensor(
                                sps[:, x * S : x * S + GN],
                                sps[:, x * S : x * S + GN],
                                gt[:], ALU.mult,
                            )
                    pt = ptpool.tile([P, 2 * S], BF16, tag="pt")
                    nc.scalar.activation(
                        pt[0:rows, :], sps[0:rows, :], FT.Exp,
                        bias=maskb[0:rows, ci : ci + 1], scale=0.125,
                    )
                    tiles[ci] = pt
                return tiles

            def emit_av(t, ptiles):
                lrec = rlpool.tile([1, 2 * S], F32, tag="lrec")
                for x in range(2):
                    h = 2 * t + x
                    ops = apsum.tile([P, S], F32, tag="ap")
                    for ci, (off, rows) in enumerate(KCH):
                        nc.tensor.matmul(
                            ops[0:EH, :], vha[ci][0:rows, ds(h * EH, EH)],
                            ptiles[ci][0:rows, ts(x, S)],
                            start=(ci == 0), stop=(ci == NKC - 1),
                        )
                    lsb_ = rlpool.tile([1, S], F32, tag="lsb")
                    nc.vector.tensor_copy(lsb_[:], ops[DK : DK + 1, :])
                    nc.vector.reciprocal_approx_fast(
                        lrec[0:1, ts(x, S)], lsb_[0:1, :]
                    )
                    nc.vector.tensor_copy(
                        oT[t][x * DK : (x + 1) * DK, :], ops[0:DK, :]
                    )
                rlr = rlbpool.tile([1, 2 * S], mybir.dt.float32r, tag="rlr")
                nc.vector.tensor_copy(rlr[:], lrec[:])
                lba = apsum.tile([P, S], F32, tag="ap", name=f"lba{t}")
                nc.tensor.matmul(
                    lba[0:DK, :], ones64[:], rlr[0:1, 0:S], start=True, stop=True
                )
                lbb = apsum.tile([P, S], F32, tag="ap", name=f"lbb{t}")
                nc.tensor.matmul(
                    lbb[0:DK, :], ones64[:], rlr[0:1, ts(1, S)], start=True, stop=True
                )
                oa = oT[t][0:DK, :]
                nc.vector.tensor_tensor(oa, oa, lba[0:DK, :], ALU.mult)
                ob = oT[t][DK:P, :]
                nc.vector.tensor_tensor(ob, ob, lbb[0:DK, :], ALU.mult)

            # ---- merge helpers: k_i 0..5 accumulate early, 6..7 close late ----
            out_view = outT.rearrange("(t p) f -> t p f", p=P)
            mps = {}

            def merge_start(m):
                if m % 2 == 0:
                    ps = ppsum.tile([P, S], F32, tag="pp", name=f"mps{m}")
                else:
                    ps = spsum.tile([P, 2 * S], F32, tag="sp", name=f"mps{m}")[:, 0:S]
                for k_i in range(NDT - 2):
                    nc.tensor.matmul(
                        ps[:], wmt[k_i][:, ts(m, P)], oT[k_i][:],
                        start=(k_i == 0), stop=False,
                    )
                mps[m] = ps

            def merge_fin(m):
                ps = mps.pop(m)
                for k_i in (NDT - 2, NDT - 1):
                    nc.tensor.matmul(
                        ps[:], wmt[k_i][:, ts(m, P)], oT[k_i][:],
                        start=False, stop=(k_i == NDT - 1),
                    )
                ot = outpool.tile([P, S], BF16, tag="out")
                if m % 2 == 0:
                    nc.scalar.activation(
                        ot[:], ps[:], FT.Identity, bias=bmt[:, m : m + 1]
                    )
                else:
                    nc.vector.tensor_scalar_add(ot[:], ps[:], bmt[:, m : m + 1])
                out_eng = (nc.gpsimd, nc.sync, nc.scalar)[m % 3]
                out_eng.dma_start(out_view[m], ot[:])

            # ---- main interleaved loop ----
            prev = None
            for t in range(NPAIR):
                emit_qproj(t)
                emit_kproj(t)
                cur = emit_scores(t)
                if prev is not None:
                    emit_av(t - 1, prev)
                prev = cur
            merge_start(0)
            emit_av(NPAIR - 1, prev)
            merge_start(1)
            merge_start(2)
            merge_start(3)
            for m in range(NDT):
                merge_fin(m)
                if m + 4 < NDT:
                    merge_start(m + 4)

    nc.compile()
    return nc


def _get_module():
    if "nc" not in _CACHE:
        _CACHE["nc"] = _build_module()
    return _CACHE["nc"]


def _bf16(x: np.ndarray) -> np.ndarray:
    return np.ascontiguousarray(x, dtype=np.float32).astype(ml_dtypes.bfloat16)


def kernel(q, k, v, mask, graph, Wv, bv, Wk, bk, Wq, bq, Wm, bm, _trace=False):
    nc = _get_module()
    q = np.asarray(q, np.float32)
    k = np.asarray(k, np.float32)
    v = np.asarray(v, np.float32)
    mask = np.asarray(mask)
    graph = np.asarray(graph, np.float32)
    Wm_ = np.asarray(Wm, np.float32)
    bm_ = np.asarray(bm, np.float32) + Wm_ @ np.asarray(bv, np.float32)

    shared = {
        "wqT": _bf16(np.asarray(Wq, np.float32).T),
        "wkT": _bf16(np.asarray(Wk, np.float32).T),
        "wvT": _bf16(np.asarray(Wv, np.float32).T),
        "wmT": _bf16(Wm_.T),
        "bq": np.ascontiguousarray(np.asarray(bq, np.float32).reshape(NDT, P).T),
        "bk": np.ascontiguousarray(np.asarray(bk, np.float32).reshape(NDT, P).T),
        "bm": np.ascontiguousarray(bm_.reshape(NDT, P).T),
    }
    eye = np.eye(GN, dtype=np.float32)
    in_maps = []
    for b in range(B):
        idx = np.where(~mask[b, 0, 0])[0]
        n = len(idx)
        assert n <= KP, f"unmasked key count {n} exceeds capacity {KP}"
        kg = np.zeros((KP, D), np.float32)
        kg[:n] = k[b][idx]
        vg = np.zeros((KP, D), np.float32)
        vg[:n] = v[b][idx]
        mb = np.full(KP, -1e9, np.float32)
        mb[:n] = 0.0
        mbt = np.zeros((P, NKC), np.float32)
        for ci, (off, rows) in enumerate(KCH):
            mbt[:rows, ci] = mb[off : off + rows]
            mbt[rows:, ci] = -1e9
        n_g = int((idx < GN).sum())
        gfull = graph[b] + eye
        gtile = np.ones((P, GN), np.float32)
        gtile[:n_g] = gfull[:, idx[:n_g]].T
        in_maps.append(
            dict(
                shared,
                qinT=_bf16(q[b].T),
                kinT=_bf16(kg.T),
                vinT=_bf16(vg.T),
                maskb=mbt,
                gT=gtile,
            )
        )

    res = bass_utils.run_bass_kernel_spmd(
        nc, in_maps, core_ids=list(range(B)), trace=_trace
    )
    out = np.stack([r["outT"].T.astype(np.float32) for r in res.results])
    if _trace:
        kernel._last_results = res
    return out
